# revision 68
# baseline (speedup 1.0000x reference)
"""Multi-head self-attention TRN2 kernel (data-parallel over batch).

Problem: B=8, S=1024, D=384, H=8, per-head full D->D projections,
causal + key-padding mask, softmax, out_linear (H*D)->D, query-mask output.

Sharding: batch b -> NeuronCore b (8 cores, no collectives).

Algebraic restructure (host precompute, exact):
  M_h = Wq_h @ Wk_h^T   ->  scores_raw = x M_h x^T   (K-projection eliminated)
  N_h = Wv_h @ Wo_h     ->  out = sum_h softmax(scores) @ (x N_h)  (out-proj eliminated)
  bias folds: Q.bk term is constant per query row -> cancels in softmax;
  bq.K term -> per-key exp bias column; (attn@bv)Wo = bv@Wo -> folded into bo.

Dual packing: the same mask gates keys AND queries (and the output rows), so
the host packs the valid positions once (padded count Kp shared across cores,
zero columns beyond). Both sides of attention use the packed x; causality in
packed space is exactly triangular in packed indices (positions are sorted),
so the causal masks are the same static affine patterns as the dense kernel.
Pad keys are killed via the per-key exp bias (-120 -> exp 0). The packed
output rows are scattered back to original positions with 0/1 permutation
matmuls (f32r, exact); masked rows come out as exact zeros.

Per-core dataflow (one batch element), transpose-free, bf16 matmuls:
  For each head h:
    P1: Q'T[e,i] = M-chunks @ xk  (packed queries)
    P2: U[j, e|1] = xk-chunks @ N, col 384 = ones
    per packed q group (256,256,128), key chunks j <= group end:
      P3: scoresT[j,i] psum = xk-chunk stationary @ Q'T
      diagonal chunks: min(scores, static affine pattern) in-psum (DVE)
      attnT = exp(scores*inv_sqrt_d + keybias[j]) -> bf16 SBUF (ACT)
      P4 per packed q-tile: psum[i, 0:385] = sum_j attnT-chunk stat @ U
        col 384 = colsum -> recip (DVE) -> out_acc[i,:] += psum*recip (STT)
  unpack: out[s,:] = sum_slots Perm[slot]^T @ out_acc-chunk  (f32r) -> DRAM
"""

import os
from contextlib import ExitStack

import numpy as np

B, S, D, H = 8, 1024, 384, 8
P = 128
DC = D // P          # 3 partition chunks of the d/e axes
NQT = S // P         # 8 original s tiles of 128
GW = 256             # packed q-group width
DU = D + 1           # U width incl. ones column for in-P4 colsum
BIG = 3.0e38
INV_SQRT_D = float(1.0 / np.sqrt(np.float32(D), dtype=np.float32))
KNEG = -120.0                      # exp bias for pad keys -> exp==0 in bf16
RAWNEG = float(KNEG / INV_SQRT_D)  # raw-score causal fill; scaled -> -120
TPAD = 1 << 20                     # pseudo-position for pad entries

CFG = {"dt": os.environ.get("MHA_DT", "bf16")}

_BUILT = None  # (nc, cfg, plan_key)


def _dt(kind):
    import concourse.mybir as mybir

    return {"bf16": mybir.dt.bfloat16, "f32r": mybir.dt.float32r,
            "f32": mybir.dt.float32}[kind]


def _np_dt(kind):
    import ml_dtypes

    return ml_dtypes.bfloat16 if kind == "bf16" else np.float32


def plan_from_mask(mask):
    """Packed-position table and the static unpack slot list.

    tj [B, Kp]: sorted original positions of valid entries (TPAD pads).
    slots: list of (orig_tile qt, packed_chunk c) pairs covering, for every
    core, all packed chunks whose positions fall in orig tile qt.
    """
    m = np.asarray(mask) != 0
    counts = m.sum(axis=1)
    Kp = int(-(-counts.max() // P) * P)
    NK = Kp // P
    tj = np.full((B, Kp), TPAD, np.int64)
    for b in range(B):
        idx = np.nonzero(m[b])[0]
        tj[b, : len(idx)] = idx
    slots = []
    for qt in range(NQT):
        cs = set()
        for b in range(B):
            lo = int(np.searchsorted(tj[b], qt * P))
            hi = int(np.searchsorted(tj[b], qt * P + P))
            for c in range(lo // P, max(lo // P, (hi - 1) // P) + 1):
                if c < NK and lo < hi:
                    cs.add(c)
        for c in sorted(cs):
            slots.append((qt, c))
    return {"tj": tj, "Kp": Kp, "NK": NK, "slots": slots}


def _plan_key(plan):
    return (plan["Kp"], tuple(plan["slots"]))


def build(cfg=None, plan=None):
    import concourse.bass as bass
    import concourse.bacc as bacc
    import concourse.tile as tile
    import concourse.mybir as mybir

    assert plan is not None
    cfg = dict(CFG if cfg is None else cfg)
    f32 = mybir.dt.float32
    f32r = mybir.dt.float32r
    dt = _dt(cfg["dt"])
    NK, Kp, slots = plan["NK"], plan["Kp"], plan["slots"]
    NSL = len(slots)
    # packed q groups: (start, width); chunk j live iff j*P < start+width
    groups = []
    g0 = 0
    while g0 < Kp:
        w = min(GW, Kp - g0)
        groups.append((g0, w))
        g0 += w

    nc = bacc.Bacc("TRN2", target_bir_lowering=False, debug=False)

    xk_d = nc.dram_tensor("xk", [D, Kp], dt, kind="ExternalInput")
    m_d = nc.dram_tensor("M", [H, D, D], dt, kind="ExternalInput")
    n_d = nc.dram_tensor("N", [H, D, D], dt, kind="ExternalInput")
    kb_d = nc.dram_tensor("kbT", [P, NK], f32, kind="ExternalInput")
    bo_d = nc.dram_tensor("bo", [P, D], f32, kind="ExternalInput")
    perm_d = nc.dram_tensor("perm", [NSL, P, P], f32r, kind="ExternalInput")
    out_d = nc.dram_tensor("out", [S, D], f32, kind="ExternalOutput")

    with tile.TileContext(nc) as tc, ExitStack() as ctx:
        consts = ctx.enter_context(tc.tile_pool(name="consts", bufs=1))
        wpool = ctx.enter_context(tc.tile_pool(name="wpool", bufs=2))
        qpool = ctx.enter_context(tc.tile_pool(name="qpool", bufs=2))
        upool = ctx.enter_context(tc.tile_pool(name="upool", bufs=2))
        apool = ctx.enter_context(tc.tile_pool(name="apool", bufs=3))
        small = ctx.enter_context(tc.tile_pool(name="small", bufs=16))
        opool = ctx.enter_context(tc.tile_pool(name="opool", bufs=2))
        ps_pj = ctx.enter_context(tc.tile_pool(name="ps_pj", bufs=2, space="PSUM"))
        ps_sc = ctx.enter_context(tc.tile_pool(name="ps_sc", bufs=3, space="PSUM"))
        ps_pv = ctx.enter_context(tc.tile_pool(name="ps_pv", bufs=3, space="PSUM"))

        # ---- setup: head-0 weights and packed x first (P1 is the warmup)
        wtiles = {}

        def _alloc_w(h):
            ms = [
                wpool.tile([P, D], dt, tag=f"m{dc}", name=f"m{dc}")
                for dc in range(DC)
            ]
            ns = [
                wpool.tile([P, D], dt, tag=f"n{dc}", name=f"n{dc}")
                for dc in range(DC)
            ]
            wtiles[h] = (ms, ns)

        def _dma_w(h, kind, dc):
            src = m_d if kind == 0 else n_d
            nc.sync.dma_start(
                out=wtiles[h][kind][dc],
                in_=src.ap()[h, dc * P : (dc + 1) * P, :],
            )

        def _fetch_w(h):
            _alloc_w(h)
            for kind in range(2):
                for dc in range(DC):
                    _dma_w(h, kind, dc)

        xk_t = []
        _alloc_w(0)
        for dc in range(DC):
            _dma_w(0, 0, dc)
            t_ = consts.tile([P, Kp], dt, tag=f"xk{dc}", name=f"xk{dc}")
            nc.scalar.dma_start(
                out=t_, in_=xk_d.ap()[dc * P : (dc + 1) * P, :]
            )
            xk_t.append(t_)
        for dc in range(DC):
            _dma_w(0, 1, dc)
        _fetch_w(1)

        # unpack permutation blocks: needed only at the end, but the scalar
        # HWDGE queue is idle after xk, so stream them in the background now
        perm_sb = consts.tile([P, NSL, P], f32r, tag="perm")
        nc.scalar.dma_start(
            out=perm_sb, in_=perm_d.ap().rearrange("n p q -> p n q")
        )

        kb_sb = consts.tile([P, NK], f32, tag="kbT")
        nc.sync.dma_start(out=kb_sb, in_=kb_d.ap())

        bo_sb = consts.tile([P, D], f32, tag="bo")
        nc.sync.dma_start(out=bo_sb, in_=bo_d.ap())

        # causal min-mask patterns (packed space is exactly triangular):
        # keep (BIG) where i_local >= j_local + off, else RAWNEG.
        mt = []
        for off in (0, 128):
            t_ = consts.tile([P, GW], f32, tag=f"mt{off}")
            nc.vector.memset(t_, BIG)
            nc.gpsimd.affine_select(
                out=t_, in_=t_,
                compare_op=mybir.AluOpType.is_ge,
                fill=RAWNEG, base=-off, channel_multiplier=-1,
                pattern=[[1, GW]],
            )
            mt.append(t_)

        # packed out accumulator, init = bo (all packed queries are valid);
        # one contiguous tile per chunk so the f32r unpack matmul can read it
        out_accs = []
        for j in range(NK):
            t_ = consts.tile([P, D], f32r, tag=f"oa{j}", name=f"oa{j}")
            nc.vector.tensor_copy(out=t_, in_=bo_sb)
            out_accs.append(t_)

        # ---- per-head pipeline ----
        n_heads = int(os.environ.get("MHA_HEADS", str(H)))

        pending = [None]

        def _p4(gi, att_t, u_ref):
            g0, w = groups[gi]
            for qi in range(w // P):
                qt = g0 // P + qi
                ps_p = ps_pv.tile([P, DU], f32, tag="pv", name="ps_p4")
                for j in range(qt + 1):
                    nc.tensor.matmul(
                        ps_p,
                        att_t[:, j, qi * P : (qi + 1) * P],
                        u_ref[:, j, :],
                        start=(j == 0),
                        stop=(j == qt),
                    )
                guard = small.tile([P, 1], f32, tag="guard")
                nc.vector.tensor_scalar_add(
                    out=guard, in0=ps_p[:, D : D + 1], scalar1=1e-30
                )
                recip = small.tile([P, 1], f32, tag="recip")
                nc.vector.reciprocal(out=recip, in_=guard)
                nc.vector.scalar_tensor_tensor(
                    out=out_accs[qt],
                    in0=ps_p[:, :D],
                    scalar=recip,
                    in1=out_accs[qt],
                    op0=mybir.AluOpType.mult,
                    op1=mybir.AluOpType.add,
                )

        for h in range(n_heads):
            m_t, n_t = wtiles.pop(h)

            # P1: Q'T [e, packed i] in 512/remainder blocks
            qp_sb = qpool.tile([P, DC, Kp], dt, tag="QT")
            k = 0
            b0 = 0
            while b0 < Kp:
                bw = min(512, Kp - b0)
                for ec in range(DC):
                    pool = ps_pj if k % 2 == 0 else ps_sc
                    ps = pool.tile([P, 512], f32,
                                   tag="pj" if k % 2 == 0 else "sc",
                                   name="ps_p1")
                    for dc in range(DC):
                        nc.tensor.matmul(
                            ps[:, :bw],
                            m_t[dc][:, ec * P : (ec + 1) * P],
                            xk_t[dc][:, b0 : b0 + bw],
                            start=(dc == 0),
                            stop=(dc == DC - 1),
                        )
                    if k % 2 == 0:
                        nc.scalar.copy(
                            out=qp_sb[:, ec, b0 : b0 + bw], in_=ps[:, :bw]
                        )
                    else:
                        nc.vector.tensor_copy(
                            out=qp_sb[:, ec, b0 : b0 + bw], in_=ps[:, :bw]
                        )
                    k += 1
                b0 += bw

            # P2: U [j, e] over packed chunks + ones column 384
            u_sb = upool.tile([P, NK, DU], dt, tag="U")
            nc.vector.memset(u_sb[:, :, D], 1.0)
            for j in range(NK):
                psu = ps_pv.tile([P, DU], f32, tag="pv", name="ps_u")
                for dc in range(DC):
                    nc.tensor.matmul(
                        psu[:, :D],
                        xk_t[dc][:, j * P : (j + 1) * P],
                        n_t[dc],
                        start=(dc == 0),
                        stop=(dc == DC - 1),
                    )
                if j % 2 == 0:
                    nc.scalar.copy(out=u_sb[:, j, :D], in_=psu[:, :D])
                else:
                    nc.vector.tensor_copy(out=u_sb[:, j, :D], in_=psu[:, :D])

            if h + 2 < n_heads:
                _fetch_w(h + 2)

            # flush the previous head's last attention group
            if pending[0] is not None:
                _p4(*pending[0])
                pending[0] = None

            for gi in range(len(groups)):
                g0, w = groups[gi]
                njc = (g0 + w + P - 1) // P   # live key chunks
                att_t = apool.tile([P, NK, GW], dt, tag="attnT", name="att_t")
                for j in range(njc):
                    ps_s = ps_sc.tile([P, GW], f32, tag="sc")
                    for ec in range(DC):
                        nc.tensor.matmul(
                            ps_s[:, :w],
                            xk_t[ec][:, j * P : (j + 1) * P],
                            qp_sb[:, ec, g0 : g0 + w],
                            start=(ec == 0),
                            stop=(ec == DC - 1),
                        )
                    off = j * P - g0
                    if off >= 0:  # diagonal chunk
                        nc.vector.tensor_tensor(
                            out=ps_s[:, :w], in0=ps_s[:, :w],
                            in1=mt[off // P][:, :w],
                            op=mybir.AluOpType.min,
                        )
                    nc.scalar.activation(
                        out=att_t[:, j, :w],
                        in_=ps_s[:, :w],
                        func=mybir.ActivationFunctionType.Exp,
                        scale=INV_SQRT_D,
                        bias=kb_sb[:, j : j + 1],
                    )
                if pending[0] is not None:
                    _p4(*pending[0])
                pending[0] = (gi, att_t, u_sb)

        # ---- unpack: out[s,:] = sum_slots Perm[sl]^T @ out_acc chunk ----
        by_qt = {}
        for sl, (qt, c) in enumerate(slots):
            by_qt.setdefault(qt, []).append((sl, c))

        kk = [0]

        def _unpack(qts):
            for qt in qts:
                sls = by_qt.get(qt, [])
                ps_o = ps_pj.tile([P, 512], f32, tag="pj", name="ps_unpack")
                if not sls:
                    nc.vector.memset(ps_o[:, :D], 0.0)
                for i, (sl, c) in enumerate(sls):
                    nc.tensor.matmul(
                        ps_o[:, :D],
                        perm_sb[:, sl, :],
                        out_accs[c],
                        start=(i == 0),
                        stop=(i == len(sls) - 1),
                    )
                st = opool.tile([P, D], f32, tag="st")
                if kk[0] % 2 == 0:
                    nc.scalar.copy(out=st, in_=ps_o[:, :D])
                else:
                    nc.vector.tensor_copy(out=st, in_=ps_o[:, :D])
                kk[0] += 1
                nc.sync.dma_start(
                    out=out_d.ap()[qt * P : (qt + 1) * P, :], in_=st
                )

        # tiles untouched by the final group's packed chunks are already
        # final: unpack them under the last P4's exp/DVE latency
        g0l, wl = groups[-1]
        fin = set(range(g0l // P, (g0l + wl) // P))
        pre = [qt for qt in range(NQT)
               if by_qt.get(qt) and not any(c in fin for _, c in by_qt[qt])]
        post = [qt for qt in range(NQT) if qt not in pre]
        _unpack(pre)
        _p4(*pending[0])
        _unpack(post)

    nc.compile()
    return nc


def _in_maps(x, mask, Wq, bq, Wk, bk, Wv, bv, Wo, bo, cfg, plan):
    np_dt = _np_dt(cfg["dt"])
    f32 = np.float32
    x = np.asarray(x, f32)
    Wq = np.asarray(Wq, f32)
    Wk = np.asarray(Wk, f32)
    Wv = np.asarray(Wv, f32)
    Wo = np.asarray(Wo, f32).reshape(H, D, D)
    bq = np.asarray(bq, f32)
    bk = np.asarray(bk, f32)
    bv = np.asarray(bv, f32)
    bo = np.asarray(bo, f32)

    M = np.einsum("hde,hfe->hdf", Wq, Wk)
    N = np.einsum("hde,hef->hdf", Wv, Wo)
    bo_f = bo + np.einsum("hd,hdf->f", bv, Wo)

    tj, Kp, NK = plan["tj"], plan["Kp"], plan["NK"]
    slots = plan["slots"]

    shared = {
        "M": M.astype(np_dt),
        "N": N.astype(np_dt),
        "bo": np.broadcast_to(bo_f[None, :], (P, D)).copy(),
    }
    xT = np.ascontiguousarray(x.transpose(0, 2, 1))  # [B, D, S]
    maps = []
    for b in range(B):
        tjb = tj[b]
        valid = tjb < S
        xk = np.zeros((D, Kp), f32)
        xk[:, valid] = xT[b][:, tjb[valid]]
        kb = np.where(valid, 0.0, np.float32(KNEG)).astype(f32)
        perm = np.zeros((len(slots), P, P), f32)
        for sl, (qt, c) in enumerate(slots):
            pos = tjb[c * P : (c + 1) * P]            # orig position per row
            loc = pos - qt * P                        # col within orig tile
            sel = (loc >= 0) & (loc < P)
            perm[sl, np.nonzero(sel)[0], loc[sel]] = 1.0
        maps.append(
            {
                "xk": xk.astype(np_dt),
                "kbT": np.ascontiguousarray(kb.reshape(NK, P).T),
                "perm": perm,
                **shared,
            }
        )
    return maps


def run(inputs, trace=False, cfg=None):
    """inputs: dict from setup_inputs(). Returns (out [B,S,D] f32, results)."""
    from concourse.bass_utils import run_bass_kernel_spmd

    global _BUILT
    cfg = dict(CFG if cfg is None else cfg)
    plan = plan_from_mask(inputs["mask"])
    pk = _plan_key(plan)
    if _BUILT is None or _BUILT[1] != cfg or _BUILT[2] != pk:
        _BUILT = (build(cfg, plan), cfg, pk)
    nc = _BUILT[0]
    in_maps = _in_maps(**inputs, cfg=cfg, plan=plan)
    res = run_bass_kernel_spmd(
        nc, in_maps, core_ids=list(range(B)), trace=trace
    )
    out = np.stack([np.asarray(res.results[b]["out"], np.float32) for b in range(B)])
    return out, res


def kernel(**inputs):
    out, _ = run(inputs, trace=False)
    return out
